# revision 29
# baseline (speedup 1.0000x reference)
"""BiLSTM-CRF loss kernel for 8 Trainium2 NeuronCores.

Time-windowed data layout (v2):
  - The LSTM forget gates keep |df/dstate| ~ 0.5, so a 32-step zero-state
    warmup reconstructs the state to ~1e-6.  T=512 is split into 4 windows
    of 128 steps; cores 0..3 run the forward LSTM on windows 0..3, cores
    4..7 the backward LSTM (time-reversed stream) on the same windows.
    Every core carries the FULL batch of 32 and runs 160 steps
    (32 warmup + 128 window); window-initial cores get wmask=0 to reset
    state to exactly zero after the (junk) warmup.
  - Per core: embedding gather -> PE transpose -> xg precompute in two
    half-window passes (SBUF) -> 160-step recurrence (fp8 whhT stationary,
    gates in [i,i,f,f,o,o,g,g] half-chunk order so ACT/DVE overlap the PE
    stream; h written into an SBUF history) -> batched emit projection.
  - One 8-rank AllGather shares all emit halves; every core rebuilds the
    full [L, T*B] combined emit and redundantly runs the exp-domain CRF
    DP for all 32 sequences plus the golden-path mask reduction.
  - Host combines core 0's scalars with the input-only terms.
"""

import numpy as np
import ml_dtypes

import concourse.bacc as bacc
import concourse.bass as bass
import concourse.mybir as mybir
import concourse.tile as tile
from concourse.bass import IndirectOffsetOnAxis

dt = mybir.dt
F32 = dt.float32
BF16 = dt.bfloat16
F8 = dt.float8e4
I32 = dt.int32
BF = ml_dtypes.bfloat16
F8NP = ml_dtypes.float8_e4m3

V, E, H, B, T, L = 50000, 256, 512, 32, 512, 48
PAD_IDX, BOS_IDX, EOS_IDX = 0, 1, 2
RENORM = 4
WUP = 16          # warmup steps (state err ~5e-4 << bf16 h noise)
CH = 128          # window length
NW = T // CH      # 4 windows


def build_nc(Tn=T, n_cores=8, Vn=V, rec_reps=1):
    Bl = B                      # full batch per core
    Tl = WUP + CH               # 160 local steps
    NBT_l = Tl * Bl             # 5120 gathered tokens per core
    NBT_e = CH * Bl             # 4096 emit cols per core
    NBT = Tn * Bl               # 16384 total (t, b) cols
    G4 = 4 * H
    KC = H // 128               # 4
    MC = G4 // 128              # 16
    EC = E // 128               # 2
    SW = KC * Bl                # 128: per-step h/c width
    HW_ = SW // 2               # 64: half width
    HT = Tl // 2                # 80 steps per xg pass
    HTOK = HT * Bl              # 2560 tokens per xg pass
    ACT = mybir.ActivationFunctionType

    nc = bacc.Bacc("TRN2", target_bir_lowering=False, debug=False,
                   num_devices=n_cores)

    emb_d = nc.dram_tensor("emb", [Vn, E], BF16, kind="ExternalInput").ap()
    src_d = nc.dram_tensor("src", [NBT_l], I32, kind="ExternalInput").ap()
    wihT_d = nc.dram_tensor("wihT", [E, G4], BF16, kind="ExternalInput").ap()
    whhT_d = nc.dram_tensor("whhT", [H, G4], F8, kind="ExternalInput").ap()
    bias_d = nc.dram_tensor("bias", [128, MC], F32, kind="ExternalInput").ap()
    w1T_d = nc.dram_tensor("w1T", [H, L], BF16, kind="ExternalInput").ap()
    expT_d = nc.dram_tensor("expT", [L, L], F32, kind="ExternalInput").ap()
    expTT_d = nc.dram_tensor("expTT", [L, L], F32, kind="ExternalInput").ap()
    etb_d = nc.dram_tensor("etb", [L, 1], F32, kind="ExternalInput").ap()
    bemit_d = nc.dram_tensor("bemit", [L, 1], F32, kind="ExternalInput").ap()
    eosv_d = nc.dram_tensor("eosv", [L, 1], F32, kind="ExternalInput").ap()
    mask_d = nc.dram_tensor("mask", [L, NBT], BF16, kind="ExternalInput").ap()
    ident_d = nc.dram_tensor("ident", [128, 128], BF16,
                             kind="ExternalInput").ap()
    wmask_d = nc.dram_tensor("wmask", [128, 1], F32, kind="ExternalInput").ap()
    out_d = nc.dram_tensor("out", [1, 8], F32, kind="ExternalOutput").ap()

    emitF_sh = nc.dram_tensor("emitF_sh", [L, NBT_e], BF16,
                              kind="Internal").ap()
    gath_sh = nc.dram_tensor("gath_sh", [n_cores, L, NBT_e], BF16,
                             kind="Internal").ap()

    with tile.TileContext(nc) as tc:
        with (
            tc.tile_pool(name="consts", bufs=1) as consts,
            tc.tile_pool(name="emitsb", bufs=1) as emitsb,
        ):
            whhT_sb = []
            for k in range(KC):
                t_ = consts.tile([128, G4], F8, tag=f"whhT{k}")
                nc.sync.dma_start(t_[:], whhT_d[k * 128:(k + 1) * 128, :])
                whhT_sb.append(t_)
            w1T_sb = []
            for k in range(KC):
                t_ = consts.tile([128, L], BF16, tag=f"w1T{k}")
                nc.sync.dma_start(t_[:], w1T_d[k * 128:(k + 1) * 128, :])
                w1T_sb.append(t_)
            bias_sb = consts.tile([128, MC], F32, tag="bias")
            nc.sync.dma_start(bias_sb[:], bias_d[:])
            wmask_sb = consts.tile([128, 1], F32, tag="wmask")
            nc.sync.dma_start(wmask_sb[:], wmask_d[:])
            emitF_sb = emitsb.tile([L, NBT_e], BF16, tag="emitF")

            # ---------------- phase 1 + recurrence ----------------
            with (
                tc.tile_pool(name="xgbuf", bufs=1) as xgbuf,
                tc.tile_pool(name="hhbuf", bufs=1) as hhbuf,
                tc.tile_pool(name="ld1a", bufs=1) as ld1a,
            ):
                xg_sb = xgbuf.tile([128, MC * HTOK], BF16, tag="xg")
                xg_v = xg_sb[:].rearrange("p (m n) -> p m n", m=MC)
                hh_sb = hhbuf.tile([128, Tl * SW], BF16, tag="hh")

                wihT_sb = []
                for k in range(EC):
                    t_ = ld1a.tile([128, G4], BF16, tag=f"wihT{k}")
                    nc.sync.dma_start(t_[:], wihT_d[k * 128:(k + 1) * 128, :])
                    wihT_sb.append(t_)
                ident_sb = ld1a.tile([128, 128], BF16, tag="ident")
                nc.sync.dma_start(ident_sb[:], ident_d[:])
                srcoff = ld1a.tile([128, NBT_l // 128], I32, tag="srcoff")
                nc.sync.dma_start(srcoff[:],
                                  src_d.rearrange("(g p) -> p g", p=128))

                def xg_pass(half):
                    tok0 = half * HTOK
                    nt_sz = 384
                    with (
                        tc.tile_pool(name="xrows", bufs=4) as xrows,
                        tc.tile_pool(name="xtb", bufs=2) as xtb,
                        tc.tile_pool(name="ps_big", bufs=2,
                                     space="PSUM") as ps_big,
                    ):
                        for j in range(HTOK // nt_sz):
                            gpt = nt_sz // 128
                            g0 = (tok0 + j * nt_sz) // 128
                            xts = [xtb.tile([128, nt_sz], BF16, tag=f"xT{k}",
                                            name=f"xT{k}")
                                   for k in range(EC)]
                            for g in range(gpt):
                                xr = xrows.tile([128, E], BF16, tag="xr")
                                nc.gpsimd.indirect_dma_start(
                                    xr[:], None, emb_d,
                                    IndirectOffsetOnAxis(
                                        ap=srcoff[:, g0 + g:g0 + g + 1],
                                        axis=0))
                                for k in range(EC):
                                    tp = ps_big.tile([128, 128], BF16,
                                                     tag="tp")
                                    nc.tensor.transpose(
                                        tp[:], xr[:, k * 128:(k + 1) * 128],
                                        ident_sb[:])
                                    nc.scalar.activation(
                                        xts[k][:, g * 128:(g + 1) * 128],
                                        tp[:], ACT.Copy)
                            for m in range(MC):
                                ps = ps_big.tile([128, nt_sz], F32, tag="xgps")
                                for k in range(EC):
                                    nc.tensor.matmul(
                                        ps[:],
                                        wihT_sb[k][:, m * 128:(m + 1) * 128],
                                        xts[k][:], start=(k == 0),
                                        stop=(k == EC - 1))
                                nc.scalar.activation(
                                    xg_v[:, m, j * nt_sz:(j + 1) * nt_sz],
                                    ps[:], ACT.Identity,
                                    bias=bias_sb[:, m:m + 1])

                def rec_steps(t0, t1):
                    for t in range(t0, t1):
                        hs = hz[:] if t == 0 else \
                            hh_sb[:, (t - 1) * SW:t * SW]
                        gps = ps_gates.tile([128, MC * Bl], F32, tag="g")
                        c_new = state.tile([128, SW], F32, tag="c")
                        for ho in range(2):
                            for m in range(8 * ho, 8 * ho + 8):
                                for k in range(KC):
                                    nc.tensor.matmul(
                                        gps[:, m * Bl:(m + 1) * Bl],
                                        whhT_sb[k][:, m * 128:(m + 1) * 128],
                                        hs[:, k * Bl:(k + 1) * Bl],
                                        start=(k == 0), stop=(k == KC - 1))
                            gb = ho * 8 * Bl
                            tl_ = t % HT
                            nc.vector.tensor_tensor(
                                gps[:, gb:gb + 8 * Bl].rearrange(
                                    "p (m b) -> p m b", m=8),
                                gps[:, gb:gb + 8 * Bl].rearrange(
                                    "p (m b) -> p m b", m=8),
                                xg_v[:, 8 * ho:8 * ho + 8,
                                     tl_ * Bl:(tl_ + 1) * Bl],
                                op=mybir.AluOpType.add)
                            sio = gtmp.tile([128, 6 * Bl], BF16, tag="sio")
                            nc.scalar.activation(sio[:],
                                                 gps[:, gb:gb + 6 * Bl],
                                                 ACT.Sigmoid)
                            tg = gtmp.tile([128, 2 * Bl], BF16, tag="tg")
                            nc.scalar.activation(
                                tg[:], gps[:, gb + 6 * Bl:gb + 8 * Bl],
                                ACT.Tanh)
                            m1 = gtmp.tile([128, HW_], F32, tag="m1")
                            nc.vector.tensor_tensor(m1[:], sio[:, 0:HW_],
                                                    tg[:],
                                                    op=mybir.AluOpType.mult)
                            fc = gtmp.tile([128, HW_], F32, tag="fc")
                            nc.vector.tensor_tensor(
                                fc[:], sio[:, HW_:2 * HW_],
                                c_sb[0][:, ho * HW_:(ho + 1) * HW_],
                                op=mybir.AluOpType.mult)
                            ch = c_new[:, ho * HW_:(ho + 1) * HW_]
                            nc.vector.tensor_tensor(ch, fc[:], m1[:],
                                                    op=mybir.AluOpType.add)
                            tc2 = gtmp.tile([128, HW_], BF16, tag="tc2")
                            nc.scalar.activation(tc2[:], ch, ACT.Tanh)
                            nc.vector.tensor_tensor(
                                hh_sb[:, t * SW + ho * HW_:
                                      t * SW + (ho + 1) * HW_],
                                sio[:, 2 * HW_:3 * HW_], tc2[:],
                                op=mybir.AluOpType.mult)
                        c_sb[0] = c_new
                        if t == WUP - 1:
                            # window-initial cores: reset state to zero
                            nc.vector.tensor_scalar_mul(
                                hh_sb[:, t * SW:(t + 1) * SW],
                                hh_sb[:, t * SW:(t + 1) * SW],
                                wmask_sb[:, 0:1])
                            cz = state.tile([128, SW], F32, tag="c")
                            nc.vector.tensor_scalar_mul(
                                cz[:], c_sb[0][:], wmask_sb[:, 0:1])
                            c_sb[0] = cz

                with (
                    tc.tile_pool(name="state", bufs=3) as state,
                    tc.tile_pool(name="gtmp", bufs=4) as gtmp,
                    tc.tile_pool(name="ps_gates", bufs=2,
                                 space="PSUM") as ps_gates,
                    tc.tile_pool(name="ps_emit", bufs=2,
                                 space="PSUM") as ps_emit,
                ):
                    hz = state.tile([128, SW], BF16, tag="hz")
                    c0 = state.tile([128, SW], F32, tag="c")
                    nc.gpsimd.memset(hz[:], 0.0)
                    nc.gpsimd.memset(c0[:], 0.0)
                    c_sb = [c0]

                    for _ in range(rec_reps):
                        xg_pass(0)
                        rec_steps(0, HT)
                        xg_pass(1)
                        rec_steps(HT, Tl)

                    # batched emit over the window steps only
                    hh_v = hh_sb[:].rearrange("p (t k b) -> p k t b",
                                              k=KC, b=Bl)
                    nt_e = 512 // Bl      # 16 steps per tile
                    for j in range(CH // nt_e):
                        eps = ps_emit.tile([L, 512], F32, tag="eps")
                        t0 = WUP + j * nt_e
                        for k in range(KC):
                            nc.tensor.matmul(
                                eps[:], w1T_sb[k][:],
                                hh_v[:, k, t0:t0 + nt_e, :],
                                start=(k == 0), stop=(k == KC - 1))
                        nc.scalar.activation(
                            emitF_sb[:, j * 512:(j + 1) * 512], eps[:],
                            ACT.Copy)

            # ---------------- exchange ----------------
            nc.sync.dma_start(emitF_sh, emitF_sb[:])
            nc.gpsimd.collective_compute(
                "AllGather", mybir.AluOpType.bypass,
                [list(range(n_cores))],
                ins=[emitF_sh], outs=[gath_sh])

            # ---------------- phase 2: CRF ----------------
            with (
                tc.tile_pool(name="ph2", bufs=1) as ph2,
                tc.tile_pool(name="dp", bufs=4) as dp,
                tc.tile_pool(name="ps_dp", bufs=1, space="PSUM") as ps_dp,
            ):
                expT_sb = ph2.tile([L, L], F32, tag="expT")
                nc.sync.dma_start(expT_sb[:], expT_d)
                expTT_sb = ph2.tile([L, L], F32, tag="expTT")
                nc.sync.dma_start(expTT_sb[:], expTT_d)
                etb_sb = ph2.tile([L, 1], F32, tag="etb")
                nc.sync.dma_start(etb_sb[:], etb_d)
                bemit_sb = ph2.tile([L, 1], F32, tag="bemit")
                nc.sync.dma_start(bemit_sb[:], bemit_d)
                eosv_sb = ph2.tile([L, 1], F32, tag="eosv")
                nc.sync.dma_start(eosv_sb[:], eosv_d)
                ones_row = ph2.tile([1, L], F32, tag="ones_row")
                nc.gpsimd.memset(ones_row[:], 1.0)
                ones_col = ph2.tile([L, 1], F32, tag="ones_col")
                nc.gpsimd.memset(ones_col[:], 1.0)

                # rebuild full combined emit [L, (t b)] from all windows
                comb_sb = ph2.tile([L, NBT], F32, tag="comb")
                gacc = dp.tile([L, NW], F32, tag="gacc")
                mask_v = mask_d.rearrange("l (w n) -> l w n", w=NW)
                with tc.tile_pool(name="gw", bufs=2) as gw:
                    for w in range(NW):
                        cw = comb_sb[:, w * NBT_e:(w + 1) * NBT_e]
                        Fw = gw.tile([L, NBT_e], BF16, tag="Fw")
                        Bw = gw.tile([L, NBT_e], BF16, tag="Bw")
                        nc.sync.dma_start(Fw[:], gath_sh[w])
                        nc.sync.dma_start(Bw[:], gath_sh[NW + w])
                        bw_ap = Bw[:]
                        bw_rev = bass.AP(bw_ap.tensor,
                                         bw_ap.offset + (CH - 1) * Bl,
                                         [bw_ap.ap[0], [-Bl, CH], [1, Bl]])
                        nc.vector.tensor_tensor(
                            cw.rearrange("p (t b) -> p t b", t=CH),
                            Fw[:].rearrange("p (t b) -> p t b", t=CH),
                            bw_rev, op=mybir.AluOpType.add)
                        # golden emit score via one-hot mask, per window
                        msk = gw.tile([L, NBT_e], BF16, tag="msk")
                        nc.sync.dma_start(msk[:], mask_v[:, w, :])
                        gscr = gw.tile([L, NBT_e], F32, tag="gscr")
                        nc.vector.tensor_tensor(
                            gscr[:], comb_sb[:, w * NBT_e:(w + 1) * NBT_e],
                            msk[:], op=mybir.AluOpType.mult)
                        nc.vector.tensor_reduce(gacc[:, w:w + 1], gscr[:],
                                                axis=mybir.AxisListType.X,
                                                op=mybir.AluOpType.add)
                gld = dp.tile([L, 1], F32, tag="gld")
                nc.vector.tensor_reduce(gld[:], gacc[:],
                                        axis=mybir.AxisListType.X,
                                        op=mybir.AluOpType.add)
                gps1 = ps_dp.tile([1, 1], F32, tag="gold")
                nc.tensor.matmul(gps1[:], gld[:], ones_col[:],
                                 start=True, stop=True)

                eE_sb = comb_sb
                nc.scalar.activation(eE_sb[:], comb_sb[:], ACT.Exp,
                                     bias=bemit_sb[:, 0:1])

                # meet-in-the-middle: alpha forward + beta backward, both
                # vector chains; Z = alpha_m . beta_m at m = Tn/2 - 1
                nsteps = Tn // 2 - 1
                n_ren = len([i for i in range(nsteps)
                             if i % RENORM == RENORM - 1])
                NRT = 1 + 2 * n_ren
                nrm = ph2.tile([1, NRT * Bl], F32, tag="nrm")
                ridx = [0]

                def renorm(st, tag):
                    nc.vector.tensor_copy(
                        nrm[0:1, ridx[0] * Bl:(ridx[0] + 1) * Bl],
                        st[0:1, :])
                    rn = dp.tile([1, Bl], F32, tag="rn")
                    nc.vector.reciprocal(rn[:], st[0:1, :])
                    bc = ps_dp.tile([L, Bl], F32, tag="bc")
                    nc.tensor.matmul(bc[:], ones_row[:], rn[:],
                                     start=True, stop=True)
                    ea2 = dp.tile([L, Bl], F32, tag=tag)
                    nc.vector.tensor_tensor(ea2[:], st[:], bc[:],
                                            op=mybir.AluOpType.mult)
                    ridx[0] += 1
                    return ea2

                ea = dp.tile([L, Bl], F32, tag="ea")
                nc.vector.tensor_scalar_mul(ea[:], eE_sb[:, 0:Bl],
                                            etb_sb[:, 0:1])
                ea = renorm(ea, "ea")
                # beta init: psb = b_{Tn-2} = expT' @ (eE_{Tn-1} * eosv)
                st0 = dp.tile([L, Bl], F32, tag="eb")
                nc.vector.tensor_scalar_mul(
                    st0[:], eE_sb[:, (Tn - 1) * Bl:Tn * Bl],
                    eosv_sb[:, 0:1])
                psb = ps_dp.tile([L, Bl], F32, tag="dpsb")
                nc.tensor.matmul(psb[:], expTT_sb[:], st0[:],
                                 start=True, stop=True)

                for i in range(nsteps):
                    ren = (i % RENORM == RENORM - 1)
                    t = i + 1
                    psa = ps_dp.tile([L, Bl], F32, tag="dps")
                    nc.tensor.matmul(psa[:], expT_sb[:], ea[:],
                                     start=True, stop=True)
                    sta = dp.tile([L, Bl], F32, tag="ea")
                    nc.vector.tensor_tensor(sta[:], psa[:],
                                            eE_sb[:, t * Bl:(t + 1) * Bl],
                                            op=mybir.AluOpType.mult)
                    ea = renorm(sta, "ea") if ren else sta
                    th = Tn - 3 - i
                    stb = dp.tile([L, Bl], F32, tag="eb")
                    nc.vector.tensor_tensor(
                        stb[:], psb[:],
                        eE_sb[:, (th + 1) * Bl:(th + 2) * Bl],
                        op=mybir.AluOpType.mult)
                    if ren:
                        stb = renorm(stb, "eb")
                    psb = ps_dp.tile([L, Bl], F32, tag="dpsb")
                    nc.tensor.matmul(psb[:], expTT_sb[:], stb[:],
                                     start=True, stop=True)
                assert ridx[0] == NRT

                lnN = ph2.tile([1, NRT * Bl], F32, tag="lnN")
                nc.scalar.activation(lnN[:], nrm[:], ACT.Ln)
                lnS = dp.tile([1, Bl], F32, tag="lnS")
                nc.vector.tensor_reduce(
                    lnS[:], lnN[:].rearrange("p (r b) -> p b r", b=Bl),
                    axis=mybir.AxisListType.X, op=mybir.AluOpType.add)
                dm = dp.tile([L, Bl], F32, tag="dm")
                nc.vector.tensor_tensor(dm[:], ea[:], psb[:],
                                        op=mybir.AluOpType.mult)
                zps = ps_dp.tile([1, Bl], F32, tag="zps")
                nc.tensor.matmul(zps[:], ones_col[:], dm[:],
                                 start=True, stop=True)
                lnf = dp.tile([1, Bl], F32, tag="lnf")
                nc.scalar.activation(lnf[:], zps[:], ACT.Ln)
                pathb = dp.tile([1, Bl], F32, tag="pathb")
                nc.vector.tensor_tensor(pathb[:], lnS[:], lnf[:],
                                        op=mybir.AluOpType.add)
                outs_sb = dp.tile([1, 8], F32, tag="outs")
                nc.gpsimd.memset(outs_sb[:], 0.0)
                nc.vector.tensor_reduce(outs_sb[0:1, 0:1], pathb[:],
                                        axis=mybir.AxisListType.X,
                                        op=mybir.AluOpType.add)
                nc.vector.tensor_copy(outs_sb[0:1, 1:2], gps1[:])
                nc.sync.dma_start(out_d, outs_sb[:])

    nc.compile()
    return nc


def host_inputs(src, targets, emb, Wih_f, Whh_f, b_f, Wih_b, Whh_b, b_b,
                W_emit, b_emit, trans, Tn=T, n_cores=8):
    src = np.asarray(src, np.int64)
    targets = np.asarray(targets, np.int64)
    trans = np.asarray(trans, np.float32)
    mt = float(np.max(trans))
    expT = np.exp(trans.astype(np.float64) - mt).astype(np.float32)
    etb = np.ascontiguousarray(expT[BOS_IDX, :].reshape(L, 1))
    emb16 = np.asarray(emb, np.float32).astype(BF)
    ident = np.eye(128, dtype=np.float32).astype(BF)
    W_emit = np.asarray(W_emit, np.float32)
    W1T = np.ascontiguousarray(W_emit[:, :H].T).astype(BF)
    W2T = np.ascontiguousarray(W_emit[:, H:].T).astype(BF)
    bemit = np.asarray(b_emit, np.float32).reshape(L, 1).copy()
    eosv = np.zeros((L, 1), np.float32); eosv[EOS_IDX, 0] = 1.0

    # m-chunk permutation: [i i f f o o g g] per 256-unit half
    NEWORD = [0, 1, 4, 5, 12, 13, 8, 9, 2, 3, 6, 7, 14, 15, 10, 11]

    def lstm_pack(Wih, Whh, bvec):
        def permc(a):
            return np.ascontiguousarray(
                a.reshape(a.shape[0], 16, 128)[:, NEWORD, :]
                .reshape(a.shape[0], -1))
        wihT = permc(np.asarray(Wih, np.float32).T.copy()).astype(BF)
        whhT = permc(np.asarray(Whh, np.float32).T.copy()).astype(F8NP)
        bias = np.ascontiguousarray(
            np.asarray(bvec, np.float32).reshape(-1, 128).T)[:, NEWORD]
        bias = np.ascontiguousarray(bias)
        return wihT, whhT, bias

    wihT_f, whhT_f, bias_f = lstm_pack(Wih_f, Whh_f, b_f)
    wihT_b, whhT_b, bias_b = lstm_pack(Wih_b, Whh_b, b_b)

    # full-T golden mask, identical on every core
    mask = np.zeros((L, Tn * B), np.float32)
    mask[targets.T.reshape(-1), np.arange(Tn * B)] = 1.0
    mask = mask.astype(BF)

    Tl = WUP + CH
    src_rev = src[:, ::-1]
    in_maps = []
    for core in range(n_cores):
        fwd = core < NW
        q = core % NW
        if fwd:
            lo = CH * q - WUP
            s = np.zeros((B, Tl), np.int64)
            a0 = max(0, lo)
            s[:, a0 - lo:] = src[:, a0:lo + Tl]
        else:
            lo = (T - CH * (q + 1)) - WUP
            s = np.zeros((B, Tl), np.int64)
            a0 = max(0, lo)
            s[:, a0 - lo:] = src_rev[:, a0:lo + Tl]
        s_scan = np.ascontiguousarray(s.T).reshape(-1).astype(np.int32)
        wmask = np.full((128, 1), 1.0, np.float32)
        if (fwd and q == 0) or (not fwd and q == NW - 1):
            wmask[:] = 0.0
        in_maps.append({
            "emb": emb16,
            "src": s_scan,
            "wihT": wihT_f if fwd else wihT_b,
            "whhT": whhT_f if fwd else whhT_b,
            "bias": bias_f if fwd else bias_b,
            "w1T": W1T if fwd else W2T,
            "expT": expT,
            "expTT": np.ascontiguousarray(expT.T),
            "etb": etb,
            "bemit": bemit,
            "mask": mask,
            "ident": ident,
            "eosv": eosv,
            "wmask": wmask,
        })
    return in_maps, mt


def host_combine(results, targets, trans, b_emit, mt, Tn=T, n_cores=8):
    targets = np.asarray(targets, np.int64)
    trans = np.asarray(trans, np.float64)
    b_emit = np.asarray(b_emit, np.float64)
    path_total = float(results[0]["out"][0, 0]) + B * Tn * mt
    golden_dev = float(results[0]["out"][0, 1])
    prev = np.concatenate([np.full((B, 1), BOS_IDX, np.int64),
                           targets[:, :-1]], axis=1)
    golden = golden_dev + float(b_emit[targets].sum()) + \
        float(trans[prev, targets].sum())
    return np.float32((path_total - golden) / B)


_NC_CACHE = {}


def kernel(src, lengths, targets, emb, Wih_f, Whh_f, b_f, Wih_b, Whh_b, b_b,
           W_emit, b_emit, trans):
    from concourse.bass_utils import run_bass_kernel_spmd
    if "main" not in _NC_CACHE:
        _NC_CACHE["main"] = build_nc()
    nc = _NC_CACHE["main"]
    in_maps, mt = host_inputs(src, targets, emb, Wih_f, Whh_f, b_f,
                              Wih_b, Whh_b, b_b, W_emit, b_emit, trans)
    res = run_bass_kernel_spmd(nc, in_maps, core_ids=list(range(8)))
    return host_combine(res.results, targets, trans, b_emit, mt)
